# revision 22
# baseline (speedup 1.0000x reference)
"""Trainium2 Bass kernel for CSPNetLight message-passing GNN block.

Math (per batch b, nodes i,j in [0,128), H=256, F=48, L=9):
    z1[b,i,j,:] = edge[b,i,j,:] @ We + node[b,j,:] @ Wj + node[b,i,:] @ Wi
                  + graph[b,:] @ Wg + b1
    h1  = silu(z1)
    msg = silu(h1 @ W2 + b2)
    out[b,i,:] = mean_j msg[b,i,j,:]

Sharding: data-parallel over batch, 2 graphs per NeuronCore, 8 cores.

On-chip layout is "transposed" (feature dim on partitions, (i,j) on free):
  - edge tiles transposed [j,f]->[f,j] on the PE (fp32, via identity matmul),
    then cast to bf16 during the PSUM->SBUF copy (DVE)
  - stage-1 z1T[hc, (i,j)] accumulated entirely in PSUM by the PE:
      * K=56 matmul: lhsT = [We_c (48 rows) ; pi_nat[i0:i0+8] (8 rows)],
        rhs = [edgeT (48 rows) ; one-hot i-indicator rows (8 rows)] --
        the one-hot augmentation adds the per-i pi term
      * identity matmul accumulates pjT+pg+b1 (broadcast over i)
    then silu on ACT straight out of PSUM -> h1 (bf16)
  - stage-2: z2T[h2c, (i,j)] = sum_c W2_c.T @ h1T_c (K=128 x2, bf16),
    silu+bias on ACT -> msg (bf16), mean over j via DVE tensor_reduce
  - output transposed back via PE and DMA'd out naturally (fp32).

All matmul operands are bf16 (fp32 runs 2 PE passes/matmul); PSUM stays fp32.
"""

import sys

for _p in ("/opt/trn_rl_repo",):
    if _p not in sys.path:
        sys.path.insert(0, _p)

import numpy as np

BS, N, H, L, F = 16, 128, 256, 9, 48
NCORES = 8
BPC = BS // NCORES  # batches per core
G = 8  # i's per group tile
NGRP = N // G
KA = F + G  # augmented stage-1 contraction (48 edge feats + 8 one-hot)

# silu as x*sigmoid(x) (ACT sigmoid + DVE multiply); needed for CoreSim
# (no Silu there) and as a hardware fallback.
SILU_VIA_SIGMOID = False

_CACHE: dict = {}


def _build_program():
    from contextlib import ExitStack

    import concourse.bacc as bacc
    import concourse.tile as tile
    import concourse.mybir as mybir
    from concourse.bass import MemorySpace

    f32 = mybir.dt.float32
    bf16 = mybir.dt.bfloat16
    Silu = mybir.ActivationFunctionType.Silu
    Sigm = mybir.ActivationFunctionType.Sigmoid
    AX = mybir.AxisListType.X
    ADD = mybir.AluOpType.add
    MUL = mybir.AluOpType.mult

    nc = bacc.Bacc("TRN2", target_bir_lowering=False, debug=False)

    node_d = nc.dram_tensor("node", [BPC, N, H], f32, kind="ExternalInput")
    edge_d = nc.dram_tensor("edge", [BPC, N, N, F], f32, kind="ExternalInput")
    graphT_d = nc.dram_tensor("graphT", [L, BPC], bf16, kind="ExternalInput")
    wj_d = nc.dram_tensor("Wj", [2, 128, H], bf16, kind="ExternalInput")
    wi_d = nc.dram_tensor("Wi", [2, 128, H], bf16, kind="ExternalInput")
    wg_d = nc.dram_tensor("Wg", [L, H], bf16, kind="ExternalInput")
    we_d = nc.dram_tensor("We", [F, H], bf16, kind="ExternalInput")
    w2_d = nc.dram_tensor("W2", [2, 128, H], bf16, kind="ExternalInput")
    b1T_d = nc.dram_tensor("b1T", [128, 2], f32, kind="ExternalInput")
    b2T_d = nc.dram_tensor("b2T", [128, 2], f32, kind="ExternalInput")
    id_d = nc.dram_tensor("ident", [128, 128], f32, kind="ExternalInput")
    idb_d = nc.dram_tensor("identbf", [128, 128], bf16, kind="ExternalInput")
    oh_d = nc.dram_tensor("onehot", [G, G * 128], bf16, kind="ExternalInput")
    out_d = nc.dram_tensor("out", [BPC, N, H], f32, kind="ExternalOutput")

    with tile.TileContext(nc) as tc, ExitStack() as ctx:
        const = ctx.enter_context(tc.tile_pool(name="const", bufs=1))
        perb = ctx.enter_context(tc.tile_pool(name="perb", bufs=2))
        work = ctx.enter_context(tc.tile_pool(name="work", bufs=3))
        stat = ctx.enter_context(tc.tile_pool(name="stat", bufs=1))
        pst = ctx.enter_context(
            tc.tile_pool(name="pst", bufs=2, space=MemorySpace.PSUM)
        )
        psb = ctx.enter_context(
            tc.tile_pool(name="psb", bufs=3, space=MemorySpace.PSUM)
        )

        # ---- constants ----
        ident = const.tile([128, 128], f32, tag="ident")
        nc.sync.dma_start(ident[:], id_d[:])
        identbf = const.tile([128, 128], bf16, tag="identbf")
        nc.sync.dma_start(identbf[:], idb_d[:])
        we_sb = const.tile([F, H], bf16, tag="we")
        nc.sync.dma_start(we_sb[:], we_d[:])
        wj_sb = [const.tile([128, H], bf16, tag=f"wj{k}", name=f"wj{k}") for k in range(2)]
        wi_sb = [const.tile([128, H], bf16, tag=f"wi{k}", name=f"wi{k}") for k in range(2)]
        w2_sb = [const.tile([128, H], bf16, tag=f"w2{k}", name=f"w2{k}") for k in range(2)]
        for k in range(2):
            nc.sync.dma_start(wj_sb[k][:], wj_d[k])
            nc.sync.dma_start(wi_sb[k][:], wi_d[k])
            nc.sync.dma_start(w2_sb[k][:], w2_d[k])
        wg_sb = const.tile([L, H], bf16, tag="wg")
        nc.sync.dma_start(wg_sb[:], wg_d[:])
        b1T_sb = const.tile([128, 2], f32, tag="b1T")
        nc.sync.dma_start(b1T_sb[:], b1T_d[:])
        b2T_sb = const.tile([128, 2], f32, tag="b2T")
        nc.sync.dma_start(b2T_sb[:], b2T_d[:])
        graphT_sb = const.tile([L, BPC], bf16, tag="graphT")
        nc.sync.dma_start(graphT_sb[:], graphT_d[:])

        # ---- static double-buffered tiles (manual rotation by group) ----
        # et[k]: [KA, 1024] bf16; rows 0:48 = edgeT (rewritten per group),
        # rows 48:56 = one-hot i-indicator (static).
        et_buf = [stat.tile([KA, G * 128], bf16, tag=f"et{k}", name=f"et{k}")
                  for k in range(2)]
        # aug[c][k]: [KA, 128] bf16; rows 0:48 = We_c (static), rows 48:56 =
        # pi_nat rows for the current i-group (DMA'd per group).
        aug_buf = [
            [stat.tile([KA, 128], bf16, tag=f"aug{c}{k}", name=f"aug{c}{k}")
             for k in range(2)]
            for c in range(2)
        ]
        for k in range(2):
            nc.sync.dma_start(et_buf[k][F : F + G, :], oh_d[:])
            for c in range(2):
                nc.sync.dma_start(
                    aug_buf[c][k][0:F, :], we_d[:, c * 128 : (c + 1) * 128]
                )

        # PE warm-up: ~4us of dependency-free transposes so the HAM clock
        # gate opens (K=8/8) before the real matmuls arrive.
        warm = pst.tile([128, 128], f32, tag="pt", name="warm")
        for _ in range(16):
            nc.tensor.transpose(warm[:], ident[:], ident[:])

        # ---- all per-batch precompute up front (keeps the PE dense;
        #      avoids a mid-kernel HAM re-throttle at the batch boundary) ----
        pi_nat, pjTpg4, outacc = {}, {}, {}
        for b in range(BPC):
            node_nat = perb.tile([N, H], f32, tag="node")
            nc.sync.dma_start(node_nat[:], node_d[b])
            nodeT = [perb.tile([128, 128], bf16, tag=f"nodeT{k}", name=f"nodeT{k}_{b}") for k in range(2)]
            for k in range(2):
                pt = pst.tile([128, 128], f32, tag="pt")
                nc.tensor.transpose(
                    pt[:], node_nat[:, k * 128 : (k + 1) * 128], ident[:]
                )
                nc.vector.tensor_copy(nodeT[k][:], pt[:])

            # pi_nat[i, h] = node[b] @ Wi  (natural layout, bf16 in SBUF)
            ppi = pst.tile([128, H], f32, tag="pt")
            nc.tensor.matmul(ppi[:], nodeT[0][:], wi_sb[0][:], start=True, stop=False)
            nc.tensor.matmul(ppi[:], nodeT[1][:], wi_sb[1][:], start=False, stop=True)
            pi_nat[b] = perb.tile([128, H], bf16, tag="pinat", name=f"pinat_{b}")
            nc.vector.tensor_copy(pi_nat[b][:], ppi[:])

            # pjTpg4[b][c] = (Wj.T @ node.T + graph@Wg + b1) replicated 4x
            # along free (for the identity-matmul pj accumulate, half = 4 i's)
            pjTpg4[b] = {}
            for c in range(2):
                cs = slice(c * 128, (c + 1) * 128)
                ppg = pst.tile([128, 1], f32, tag="pt")
                nc.tensor.matmul(
                    ppg[:], wg_sb[:, cs], graphT_sb[:, b : b + 1],
                    start=True, stop=True,
                )
                pgb1 = perb.tile([128, 1], f32, tag=f"pgb1{c}")
                nc.vector.tensor_add(pgb1[:], ppg[:], b1T_sb[:, c : c + 1])

                ppj = pst.tile([128, 128], f32, tag="pt")
                nc.tensor.matmul(
                    ppj[:], wj_sb[0][:, cs], nodeT[0][:], start=True, stop=False
                )
                nc.tensor.matmul(
                    ppj[:], wj_sb[1][:, cs], nodeT[1][:], start=False, stop=True
                )
                pjTpg4[b][c] = perb.tile(
                    [128, 4, 128], bf16, tag=f"pjTpg{c}", name=f"pjTpg{c}_{b}"
                )
                nc.vector.tensor_scalar_add(
                    pjTpg4[b][c][:],
                    ppj[:].unsqueeze(1).broadcast_to((128, 4, 128)),
                    pgb1[:],
                )

            outacc[b] = {
                d: perb.tile([128, 128], f32, tag=f"oacc{d}", name=f"oacc{d}_{b}")
                for d in range(2)
            }

        # ---- main loop over (batch, i-group) ----
        for b in range(BPC):
            for g in range(NGRP):
                i0 = g * G
                k = g % 2
                en = work.tile([N, G, F], f32, tag="en")
                nc.sync.dma_start(
                    en[:], edge_d[b, i0 : i0 + G].rearrange("i j f -> j i f")
                )
                # pi rows for this group into the augmented weight tiles
                for c in range(2):
                    nc.sync.dma_start(
                        aug_buf[c][k][F : F + G, :],
                        pi_nat[b][i0 : i0 + G, c * 128 : (c + 1) * 128],
                    )
                # transpose edge [j,f] -> [f,j], cast to bf16 into et 0:48
                for half in range(2):
                    ptt = pst.tile([F, 512], f32, tag="pt")
                    for il in range(4):
                        i_loc = half * 4 + il
                        nc.tensor.transpose(
                            ptt[:, il * 128 : (il + 1) * 128],
                            en[:, i_loc, :],
                            ident[:],
                        )
                    nc.vector.tensor_copy(
                        et_buf[k][0:F, half * 512 : (half + 1) * 512], ptt[:]
                    )

                h1 = {}
                for c in range(2):
                    ps1 = psb.tile([128, G * 128], f32, tag="big")
                    for half in range(2):
                        hs = slice(half * 512, (half + 1) * 512)
                        nc.tensor.matmul(
                            ps1[:, hs], aug_buf[c][k][:], et_buf[k][:, hs],
                            start=True, stop=False, skip_group_check=True,
                        )
                    for half in range(2):
                        hs = slice(half * 512, (half + 1) * 512)
                        nc.tensor.matmul(
                            ps1[:, hs], identbf[:],
                            pjTpg4[b][c][:], start=False, stop=True,
                            skip_group_check=True,
                        )
                    h1[c] = work.tile([128, G * 128], bf16, tag=f"h1{c}", name=f"h1{c}_{b}_{g}")
                    if SILU_VIA_SIGMOID:
                        zt = work.tile([128, G * 128], f32, tag=f"zt{c}")
                        nc.scalar.activation(zt[:], ps1[:], Sigm)
                        nc.vector.tensor_tensor(h1[c][:], zt[:], ps1[:], op=MUL)
                    else:
                        nc.scalar.activation(h1[c][:], ps1[:], Silu)

                for d in range(2):
                    ds = slice(d * 128, (d + 1) * 128)
                    ps2 = psb.tile([128, G * 128], f32, tag="big")
                    for half in range(2):
                        hs = slice(half * 512, (half + 1) * 512)
                        nc.tensor.matmul(
                            ps2[:, hs], w2_sb[0][:, ds], h1[0][:, hs],
                            start=True, stop=False, skip_group_check=True,
                        )
                    for half in range(2):
                        hs = slice(half * 512, (half + 1) * 512)
                        nc.tensor.matmul(
                            ps2[:, hs], w2_sb[1][:, ds], h1[1][:, hs],
                            start=False, stop=True, skip_group_check=True,
                        )
                    msg = work.tile([128, G * 128], bf16, tag=f"msg{d}", name=f"msg{d}_{b}_{g}")
                    if SILU_VIA_SIGMOID:
                        nc.scalar.activation(
                            msg[:], ps2[:], Sigm, bias=b2T_sb[:, d : d + 1]
                        )
                        nc.vector.scalar_tensor_tensor(
                            msg[:], ps2[:], b2T_sb[:, d : d + 1], msg[:],
                            op0=ADD, op1=MUL,
                        )
                    else:
                        nc.scalar.activation(
                            msg[:], ps2[:], Silu, bias=b2T_sb[:, d : d + 1]
                        )
                    nc.vector.reduce_sum(
                        outacc[b][d][:, i0 : i0 + G],
                        msg[:].rearrange("p (i j) -> p i j", i=G),
                        axis=AX,
                    )

            # ---- write back: transpose [h,i] -> [i,h], scale by 1/N ----
            for d in range(2):
                pto = pst.tile([128, 128], f32, tag="pt")
                nc.tensor.transpose(pto[:], outacc[b][d][:], ident[:])
                onat = perb.tile([128, 128], f32, tag=f"onat{d}")
                nc.vector.tensor_scalar_mul(onat[:], pto[:], 1.0 / N)
                nc.sync.dma_start(out_d[b, :, d * 128 : (d + 1) * 128], onat[:])

    nc.compile()
    return nc


def _get_program():
    if "nc" not in _CACHE:
        _CACHE["nc"] = _build_program()
    return _CACHE["nc"]


def _make_in_maps(node_embed, edge_embed, graph_embed, W1, b1, W2, b2):
    import ml_dtypes

    f = np.float32
    bf = ml_dtypes.bfloat16
    node_embed = np.asarray(node_embed, dtype=f)
    edge_embed = np.ascontiguousarray(np.asarray(edge_embed, dtype=f))
    graph_embed = np.asarray(graph_embed, dtype=f)
    W1 = np.asarray(W1, dtype=f)
    b1 = np.asarray(b1, dtype=f)
    W2 = np.asarray(W2, dtype=f)
    b2 = np.asarray(b2, dtype=f)

    Wj = np.ascontiguousarray(W1[0:H].reshape(2, 128, H).astype(bf))
    Wi = np.ascontiguousarray(W1[H : 2 * H].reshape(2, 128, H).astype(bf))
    Wg = np.ascontiguousarray(W1[2 * H : 2 * H + L].astype(bf))
    We = np.ascontiguousarray(W1[2 * H + L :].astype(bf))
    W2s = np.ascontiguousarray(W2.reshape(2, 128, H).astype(bf))
    b1T = np.ascontiguousarray(b1.reshape(2, 128).T)
    b2T = np.ascontiguousarray(b2.reshape(2, 128).T)
    ident = np.eye(128, dtype=f)
    identbf = np.eye(128).astype(bf)
    onehot = np.zeros((G, G * 128), dtype=bf)
    for r in range(G):
        onehot[r, r * 128 : (r + 1) * 128] = bf(1.0)

    in_maps = []
    for c in range(NCORES):
        bs = slice(c * BPC, (c + 1) * BPC)
        in_maps.append(
            {
                "node": np.ascontiguousarray(node_embed[bs]),
                "edge": np.ascontiguousarray(edge_embed[bs]),
                "graphT": np.ascontiguousarray(graph_embed[bs].T.astype(bf)),
                "Wj": Wj,
                "Wi": Wi,
                "Wg": Wg,
                "We": We,
                "W2": W2s,
                "b1T": b1T,
                "b2T": b2T,
                "ident": ident,
                "identbf": identbf,
                "onehot": onehot,
            }
        )
    return in_maps


def _install_ntff_shim():
    """Provide antenv.axon_hooks for run_bass_kernel_spmd(trace=True).

    This agent image lacks antenv.axon_hooks; replicate trn_boot.py's
    ctypes NTFF hook against the injected libaxon_pjrt.so.
    """
    import types
    import ctypes
    import contextlib

    try:
        from antenv.axon_hooks import get_axon_ntff_profile_hook  # noqa: F401

        return
    except ImportError:
        pass

    so_path = "/opt/axon/libaxon_pjrt.so"
    lib = ctypes.CDLL(so_path)
    if not hasattr(lib, "axon_start_nrt_profile"):
        return
    lib.axon_start_nrt_profile.argtypes = [
        ctypes.POINTER(ctypes.c_int64),
        ctypes.c_size_t,
    ]
    lib.axon_start_nrt_profile.restype = ctypes.c_int64
    lib.axon_stop_nrt_profile.argtypes = [ctypes.c_char_p]
    lib.axon_stop_nrt_profile.restype = ctypes.c_int64

    @contextlib.contextmanager
    def _hook(output_dir, device_ids):
        import jax

        jax.devices()
        if device_ids:
            ids = (ctypes.c_int64 * len(device_ids))(*device_ids)
            rc = lib.axon_start_nrt_profile(ids, len(device_ids))
        else:
            rc = lib.axon_start_nrt_profile(None, 0)
        if rc != 0:
            raise RuntimeError(f"axon_start_nrt_profile rc={rc}")
        try:
            yield
        finally:
            n = lib.axon_stop_nrt_profile(str(output_dir).encode())
            print(f"ntff profile: {n} file(s) written to {output_dir}")

    if "antenv" not in sys.modules:
        try:
            import antenv  # noqa: F401
        except ImportError:
            sys.modules["antenv"] = types.ModuleType("antenv")
    mod = types.ModuleType("antenv.axon_hooks")
    mod.get_axon_ntff_profile_hook = lambda: _hook
    mod.set_axon_ntff_profile_hook = lambda h: None
    sys.modules["antenv.axon_hooks"] = mod


def run(node_embed, edge_embed, graph_embed, W1, b1, W2, b2, trace=False,
        tmpdir=None):
    """Run on 8 NeuronCores; returns (output, BassKernelResults)."""
    from concourse.bass_utils import run_bass_kernel_spmd

    if trace:
        _install_ntff_shim()
    nc = _get_program()
    in_maps = _make_in_maps(
        node_embed, edge_embed, graph_embed, W1, b1, W2, b2
    )
    res = run_bass_kernel_spmd(
        nc, in_maps, core_ids=list(range(NCORES)), trace=trace, tmpdir=tmpdir
    )
    out = np.concatenate([res.results[c]["out"] for c in range(NCORES)], axis=0)
    return out, res


def kernel(node_embed, edge_embed, graph_embed, W1, b1, W2, b2):
    out, _ = run(node_embed, edge_embed, graph_embed, W1, b1, W2, b2)
    return out
